# revision 35
# baseline (speedup 1.0000x reference)
"""Trainium2 Bass kernel for nn_DocREModel (doc-level relation extraction graph pooling).

Key observation: attention only enters the model through (a) rows at the 128
mention positions and (b) rows inside the 16 link spans -- ~350 of 1024 rows per
doc -- and every use is linear in the head-summed attention.  So the host
gathers exactly those rows (pure data movement, like the one-hot matrices it
already built) and each core streams ~6.4 MB instead of ~16 MB.

Sharding (8 cores): doc b -> core pair (2b, 2b+1), each handling 6 of the 12
attention heads.  Per core:
  - seq_aug=[seq|1] is split across BOTH DMA rings (the scalar ring is ~3x
    slower than sync, so its half goes out first behind the tiny consts),
  - span blocks arrive row-major [slot, (head, c)]; five bf16 adds per block
    give the head-sum, then 16 single-shot matmuls vs the 0/1 span-membership
    matrix land u^T[c,k] in disjoint slices of ONE PSUM bank (no accumulation
    chains -- interleaved chains within a bank are broken), DVE masks and
    combines, and 8 single-shot col-positioned matmuls (4 concurrent per bank
    via tile_position) give per-ct link numerators v_ct,
  - the mention block arrives host-TRANSPOSED as [c-part, (ct-half, head, ct,
    mention)] in two ct-half DMAs; five adds per half feed the mention-context
    PSUM chain,
  - a dense dummy-matmul chain pegged to the last span block warms the PE HAM
    clock right before the heavy chains (it writes a v bank that the real
    single-shot matmuls wipe).
The host adds partials across the core pair and the v row-groups, applies the
tiny normalizations, gathers mention embeddings from sequence_output (exact),
and does the logsumexp entity pooling.
"""

import os
import sys

for _p in ("/opt/trn_rl_repo", "/root/.axon_site/_ro/trn_rl_repo"):
    if os.path.isdir(_p) and _p not in sys.path:
        sys.path.insert(0, _p)

import numpy as np

B, L, H, NH = 4, 1024, 768, 12
E, MPE, K = 32, 4, 16
EM = E * MPE              # 128 mentions per doc
TYPE_DIM = 20
OFFSET = 1
HPG = NH // 2             # heads per core (2 cores per doc)
HH = HPG // 2             # heads per half (3)
RC = L // 128             # 8 chunks of 128 along c
HC = RC // 2              # ct chunks per half (4)
HA = H + 4                # hidden + ones column + pad to 772
N0 = 512                  # first PSUM bank width for the HA-dim matmuls
N1 = HA - N0              # 260
NWARM = 12                # dummy matmuls in the PE pre-warm chain
VTILE = True              # col-positioned concurrent v matmuls


def _build_nc(nsp, debug=False):
    """nsp = number of 128-row span blocks (global max over docs)."""
    import concourse.bass as bass
    import concourse.mybir as mybir
    import concourse.tile as tile
    from concourse import bacc

    f32 = mybir.dt.float32
    bf16 = mybir.dt.bfloat16
    ts, ds = bass.ts, bass.ds

    nc = bacc.Bacc("TRN2", target_bir_lowering=False, debug=debug)

    f8 = mybir.dt.float8e4
    gsp = nc.dram_tensor("gsp", [nsp * 128, HPG * L], f8, kind="ExternalInput")
    ident = nc.dram_tensor("ident", [128, 128], f8, kind="ExternalInput")
    gmt = nc.dram_tensor("gmt", [128, HPG * RC * EM], bf16, kind="ExternalInput")
    seqp = nc.dram_tensor("seqp", [128, RC * HA], bf16, kind="ExternalInput")
    wmsk = nc.dram_tensor("wmsk", [128, nsp * K + RC * K], bf16, kind="ExternalInput")
    VR = 128 if VTILE else K
    out_va = nc.dram_tensor("out_va", [VR, HA], bf16, kind="ExternalOutput")
    out_vb = nc.dram_tensor("out_vb", [VR, HA], bf16, kind="ExternalOutput")
    out_mnum = nc.dram_tensor("out_mnum", [EM, HA], bf16, kind="ExternalOutput")

    MHB = HPG * HC * EM       # mention cols per ct-half (3072)

    with tile.TileContext(nc) as tc:
        with (
            tc.tile_pool(name="const", bufs=1) as constp,
            tc.tile_pool(name="stream", bufs=max(3, nsp + 1)) as streamp,
            tc.tile_pool(name="tmp", bufs=2) as tmpp,
            tc.tile_pool(name="acc", bufs=1) as accp,
            tc.tile_pool(name="stage", bufs=1) as stagep,
            tc.tile_pool(name="psall", bufs=8, space="PSUM") as psall,
        ):
            # ---- scalar ring (slow): tiny consts, then the late seq half ----
            ident_s = constp.tile([128, 128], f8, name="idents")
            nc.scalar.dma_start(out=ident_s[:], in_=ident[:])
            wmsk_s = constp.tile([128, nsp * K + RC * K], bf16, name="wmsks")
            nc.scalar.dma_start(out=wmsk_s[:], in_=wmsk[:])
            seq_s = constp.tile([128, RC * HA], bf16, name="seqs")
            nc.scalar.dma_start(out=seq_s[:, HC * HA:], in_=seqp[:, HC * HA:])
            mc_of = nsp * K            # maskc block offset inside wmsk

            # ---- sync ring: early seq half, span blocks, mention ct-halves ----
            nc.sync.dma_start(out=seq_s[:, 0:HC * HA], in_=seqp[:, 0:HC * HA])
            gsp_t = [streamp.tile([128, HPG * L], f8, tag="gsp", name=f"gsp{sc}")
                     for sc in range(nsp)]
            gmt_t = streamp.tile([128, HPG * RC * EM], bf16, tag="gmt", name="gmt")
            for sc in range(nsp - 1):
                nc.sync.dma_start(out=gsp_t[sc][:], in_=gsp[ts(sc, 128), :])
            nc.sync.dma_start(out=gmt_t[:, 0:MHB], in_=gmt[:, 0:MHB])
            nc.sync.dma_start(out=gmt_t[:, MHB:], in_=gmt[:, MHB:])
            nc.sync.dma_start(out=gsp_t[nsp - 1][:], in_=gsp[ts(nsp - 1, 128), :])

            # ---- span head-sums on the PE: identity-matmul accumulation over the
            #      six heads (fp8 stream; PE idles during the stream anyway and
            #      this keeps its HAM clock warm -- replaces the dummy chain) ----
            hss = []
            for sc in range(nsp):
                h = accp.tile([128, L], bf16, tag=f"hss{sc}", name=f"hss{sc}")
                for half in range(2):
                    phs = psall.tile([128, N0], f32, tag="ps", name=f"hs{sc}{half}")
                    for hh in range(HPG):
                        nc.tensor.matmul(phs[:], ident_s[:],
                                         gsp_t[sc][:, ds(hh * L + half * N0, N0)],
                                         start=(hh == 0), stop=(hh == HPG - 1))
                    nc.vector.tensor_copy(h[:, ds(half * N0, N0)], phs[:])
                hss.append(h)

            # ---- u^T[c,k]: 16 single-shot matmuls into ONE psum bank ----
            pgs = psall.tile([128, nsp * RC * K], f32, tag="ps", name="pgs")
            pva0 = psall.tile([VR, N0], f32, tag="ps", name="pva0")
            pva1 = psall.tile([VR, N1], f32, tag="ps", name="pva1")
            pvb0 = psall.tile([VR, N0], f32, tag="ps", name="pvb0")
            pvb1 = psall.tile([VR, N1], f32, tag="ps", name="pvb1")

            for sc in range(nsp):
                for ct in range(RC):
                    nc.tensor.matmul(pgs[:, ds(sc * RC * K + ct * K, K)],
                                     hss[sc][:, ts(ct, 128)],
                                     wmsk_s[:, ds(sc * K, K)], start=True, stop=True)

            # ---- mask + combine span blocks on DVE ----
            wv = accp.tile([128, RC * K], bf16, tag="wv", name="wv")
            if nsp == 1:
                nc.vector.tensor_mul(wv[:], pgs[:], wmsk_s[:, ds(mc_of, RC * K)])
            else:
                wparts = []
                for sc in range(nsp):
                    w = tmpp.tile([128, RC * K], bf16, tag="wpart", name=f"w{sc}")
                    nc.vector.tensor_mul(w[:], pgs[:, ds(sc * RC * K, RC * K)],
                                         wmsk_s[:, ds(mc_of, RC * K)])
                    wparts.append(w)
                nc.vector.tensor_add(wv[:], wparts[0][:], wparts[1][:])
                for sc in range(2, nsp):
                    nc.vector.tensor_add(wv[:], wv[:], wparts[sc][:])

            # ---- v: per-ct single-shot matmuls; 4 concurrent col-groups/bank ----
            if VTILE:
                for ct in range(RC):
                    p0, p1 = (pva0, pva1) if ct < HC else (pvb0, pvb1)
                    q = ct % HC
                    nc.tensor.matmul(p0[32 * q:32 * q + K, :], wv[:, ts(ct, K)],
                                     seq_s[:, ds(ct * HA, N0)], start=True, stop=True,
                                     tile_position=(0, 32 * q))
                    nc.tensor.matmul(p1[32 * q:32 * q + K, :], wv[:, ts(ct, K)],
                                     seq_s[:, ds(ct * HA + N0, N1)], start=True, stop=True,
                                     tile_position=(0, 32 * q))
            else:
                for ct in range(RC):
                    p0, p1 = (pva0, pva1) if ct < HC else (pvb0, pvb1)
                    j = ct % HC
                    nc.tensor.matmul(p0[:], wv[:, ts(ct, K)], seq_s[:, ds(ct * HA, N0)],
                                     start=(j == 0), stop=(j == HC - 1))
                    nc.tensor.matmul(p1[:], wv[:, ts(ct, K)], seq_s[:, ds(ct * HA + N0, N1)],
                                     start=(j == 0), stop=(j == HC - 1))
            va_s = stagep.tile([VR, HA], bf16, tag="vas", name="vas")
            vb_s = stagep.tile([VR, HA], bf16, tag="vbs", name="vbs")
            nc.scalar.copy(out=va_s[:, 0:N0], in_=pva0[:])
            nc.scalar.copy(out=va_s[:, N0:HA], in_=pva1[:])
            nc.scalar.dma_start(out=out_va[:], in_=va_s[:])
            nc.scalar.copy(out=vb_s[:, 0:N0], in_=pvb0[:])
            nc.scalar.copy(out=vb_s[:, N0:HA], in_=pvb1[:])
            nc.scalar.dma_start(out=out_vb[:], in_=vb_s[:])

            # ---- mention branch: ct-half blocks, adds then PSUM chain ----
            pmn0 = psall.tile([EM, N0], f32, tag="ps", name="pmn0")
            pmn1 = psall.tile([EM, N1], f32, tag="ps", name="pmn1")
            mnum_s = stagep.tile([EM, HA], bf16, tag="mns", name="mns")
            for cth in range(2):
                base = cth * MHB
                m01 = tmpp.tile([128, HC * EM], bf16, tag="ma", name="m01")
                m23 = tmpp.tile([128, HC * EM], bf16, tag="mb", name="m23")
                m45 = tmpp.tile([128, HC * EM], bf16, tag="mc", name="m45")
                hsm = accp.tile([128, HC * EM], bf16, tag=f"hsm{cth}", name=f"hsm{cth}")
                HB = HC * EM
                nc.vector.tensor_add(m01[:], gmt_t[:, ds(base + 0 * HB, HB)],
                                     gmt_t[:, ds(base + 1 * HB, HB)])
                nc.vector.tensor_add(m23[:], gmt_t[:, ds(base + 2 * HB, HB)],
                                     gmt_t[:, ds(base + 3 * HB, HB)])
                nc.vector.tensor_add(m45[:], gmt_t[:, ds(base + 4 * HB, HB)],
                                     gmt_t[:, ds(base + 5 * HB, HB)])
                nc.vector.tensor_add(m01[:], m01[:], m23[:])
                nc.vector.tensor_add(hsm[:], m01[:], m45[:])
                for j in range(HC):
                    ct = cth * HC + j
                    nc.tensor.matmul(pmn0[:], hsm[:, ts(j, EM)], seq_s[:, ds(ct * HA, N0)],
                                     start=(ct == 0), stop=(ct == RC - 1))
                    nc.tensor.matmul(pmn1[:], hsm[:, ts(j, EM)], seq_s[:, ds(ct * HA + N0, N1)],
                                     start=(ct == 0), stop=(ct == RC - 1))
            nc.vector.tensor_copy(mnum_s[:, 0:N0], pmn0[:])
            nc.vector.tensor_copy(mnum_s[:, N0:HA], pmn1[:])
            nc.sync.dma_start(out=out_mnum[:], in_=mnum_s[:])

    nc.compile()
    return nc


_NC_CACHE = {}


def _get_nc(nsp=2):
    if nsp not in _NC_CACHE:
        _NC_CACHE[nsp] = _build_nc(nsp)
    return _NC_CACHE[nsp]


def _per_core_inputs(sequence_output, attention, mention_pos, link_start, link_len):
    """Returns (in_maps for 8 cores, per-doc span lengths, nsp)."""
    import ml_dtypes
    bf16 = ml_dtypes.bfloat16
    f8 = ml_dtypes.float8_e4m3
    identity = np.eye(128, dtype=np.float32).astype(f8)
    seq = np.asarray(sequence_output, dtype=np.float32)
    att = np.asarray(attention)
    mpos = np.asarray(mention_pos).astype(np.int64)
    lstart = np.asarray(link_start).astype(np.int64)
    llen = np.asarray(link_len).astype(np.int64)

    doc = []
    max_u = 1
    for b in range(B):
        pos = (mpos[b] + OFFSET).reshape(EM)
        s = lstart[b] + OFFSET
        e = lstart[b] + llen[b] + 1 + OFFSET
        srows = np.unique(np.concatenate([np.arange(si, ei) for si, ei in zip(s, e)]))
        max_u = max(max_u, len(srows))
        doc.append((pos, s, e, srows))
    nsp = (max_u + 127) // 128

    in_maps = []
    lengths = []
    for b in range(B):
        pos, s, e, srows = doc[b]
        nsr = len(srows)
        srows_p = np.zeros(nsp * 128, np.int64)
        srows_p[:nsr] = srows
        wspm = np.zeros((nsp * 128, K), np.float32)
        wspm[:nsr] = ((srows[:, None] >= s[None, :]) & (srows[:, None] < e[None, :]))
        wsp_p = wspm.reshape(nsp, 128, K).transpose(1, 0, 2).reshape(128, nsp * K)
        r = np.arange(L)
        maskc = ((r[:, None] >= s[None, :]) & (r[:, None] < e[None, :])).astype(np.float32)
        maskc_p = maskc.reshape(RC, 128, K).transpose(1, 0, 2).reshape(128, RC * K)
        wmsk = np.ascontiguousarray(
            np.concatenate([wsp_p, maskc_p], axis=1)).astype(bf16)
        seq_aug = np.concatenate(
            [seq[b], np.ones((L, 1), np.float32), np.zeros((L, HA - H - 1), np.float32)], axis=1)
        seqp = np.ascontiguousarray(
            seq_aug.reshape(RC, 128, HA).transpose(1, 0, 2).reshape(128, RC * HA)).astype(bf16)
        lengths.append((e - s).astype(np.float32))
        for g in range(2):
            hsl = slice(g * HPG, (g + 1) * HPG)
            # span rows, row-major: [sc*128+q, h*L+c]
            gspr = att[b, hsl][:, srows_p, :]                      # [HPG, nsp*128, L]
            gspx = np.ascontiguousarray(
                gspr.transpose(1, 0, 2).reshape(nsp * 128, HPG * L)).astype(f8)
            # mention rows, transposed, ct-half major: [p, (cth, h, ctq, m)]
            gmtr = att[b, hsl][:, pos, :]                          # [HPG, EM, L]
            gmtx = np.ascontiguousarray(
                gmtr.reshape(HPG, EM, 2, HC, 128).transpose(4, 2, 0, 3, 1)
                .reshape(128, HPG * RC * EM)).astype(bf16)
            in_maps.append({"gsp": gspx, "gmt": gmtx, "seqp": seqp, "wmsk": wmsk,
                            "ident": identity})
    return in_maps, lengths, nsp


def _combine(outs, lengths, sequence_output, type_table, mention_pos):
    seq = np.asarray(sequence_output, dtype=np.float32)
    mpos = np.asarray(mention_pos).astype(np.int64)
    ttab = np.asarray(type_table, dtype=np.float32)
    type_ids = np.concatenate(
        [np.zeros(E, np.int64), np.ones(EM, np.int64), np.full(K, 2, np.int64)])
    nodes_type = ttab[type_ids]  # [E+EM+K, TYPE_DIM]

    def vsum(o):
        va = o["out_va"].astype(np.float32)
        vb = o["out_vb"].astype(np.float32)
        if va.shape[0] == K:
            return va + vb
        groups = [va[32 * q:32 * q + K] for q in range(HC)] + \
                 [vb[32 * q:32 * q + K] for q in range(HC)]
        return np.sum(groups, axis=0)

    out = np.zeros((B, E + EM + K + E + EM, H + TYPE_DIM), np.float32)
    for b in range(B):
        o0, o1 = outs[2 * b], outs[2 * b + 1]
        v = vsum(o0) + vsum(o1)
        mnum = o0["out_mnum"].astype(np.float32) + o1["out_mnum"].astype(np.float32)
        length = lengths[b]

        link_rep = v[:, :H] / (NH * length[:, None])
        m_ctx = mnum[:, :H] / (mnum[:, H:H + 1] + NH * 1e-5)
        enum = mnum.reshape(E, MPE, HA).sum(axis=1)
        e_ctx = enum[:, :H] / (enum[:, H:H + 1] + NH * MPE * 1e-5)

        pos = (mpos[b] + OFFSET).reshape(EM)
        memb = seq[b, pos]                                          # exact gather
        mg = memb.reshape(E, MPE, H)
        mmax = mg.max(axis=1)
        eemb = np.log(np.exp(mg - mmax[:, None, :]).sum(axis=1)) + mmax

        nodes_raw = np.concatenate([eemb, memb, link_rep], axis=0)  # [176,H]
        nodes = np.concatenate([nodes_raw, nodes_type], axis=1)     # [176,H+20]
        ctx = np.concatenate([e_ctx, m_ctx], axis=0)                # [160,H]
        ctx = np.concatenate([ctx, np.zeros((E + EM, TYPE_DIM), np.float32)], axis=1)
        out[b] = np.concatenate([nodes, ctx], axis=0)
    return out


def kernel(**inputs):
    from concourse.bass_utils import run_bass_kernel_spmd

    in_maps, lengths, nsp = _per_core_inputs(
        inputs["sequence_output"], inputs["attention"],
        inputs["mention_pos"], inputs["link_start"], inputs["link_len"])
    nc = _get_nc(nsp)
    res = run_bass_kernel_spmd(nc, in_maps, core_ids=list(range(8)))
    return _combine(res.results, lengths, inputs["sequence_output"],
                    inputs["type_table"], inputs["mention_pos"])


# revision 42
# speedup vs baseline: 1.1604x; 1.1604x over previous
"""Trainium2 Bass kernel for nn_DocREModel (doc-level relation extraction graph pooling).

Key observation: attention only enters the model through (a) rows at the 128
mention positions and (b) rows inside the 16 link spans -- ~350 of 1024 rows per
doc -- and every use is linear in the head-summed attention.  So the host
gathers exactly those rows (pure data movement, like the one-hot matrices it
already built) and each core streams ~6.4 MB instead of ~16 MB.

Sharding (8 cores): doc b -> core pair (2b, 2b+1), each handling 6 of the 12
attention heads.  Per core:
  - seq_aug=[seq|1] is split across BOTH DMA rings (the scalar ring is ~3x
    slower than sync, so its half goes out first behind the tiny consts),
  - span blocks arrive row-major [slot, (head, c)]; five bf16 adds per block
    give the head-sum, then 16 single-shot matmuls vs the 0/1 span-membership
    matrix land u^T[c,k] in disjoint slices of ONE PSUM bank (no accumulation
    chains -- interleaved chains within a bank are broken), DVE masks and
    combines, and 8 single-shot col-positioned matmuls (4 concurrent per bank
    via tile_position) give per-ct link numerators v_ct,
  - the mention block arrives host-TRANSPOSED as [c-part, (ct-half, head, ct,
    mention)] in two ct-half DMAs; five adds per half feed the mention-context
    PSUM chain,
  - a dense dummy-matmul chain pegged to the last span block warms the PE HAM
    clock right before the heavy chains (it writes a v bank that the real
    single-shot matmuls wipe).
The host adds partials across the core pair and the v row-groups, applies the
tiny normalizations, gathers mention embeddings from sequence_output (exact),
and does the logsumexp entity pooling.
"""

import os
import sys

for _p in ("/opt/trn_rl_repo", "/root/.axon_site/_ro/trn_rl_repo"):
    if os.path.isdir(_p) and _p not in sys.path:
        sys.path.insert(0, _p)

import numpy as np

B, L, H, NH = 4, 1024, 768, 12
E, MPE, K = 32, 4, 16
EM = E * MPE              # 128 mentions per doc
TYPE_DIM = 20
OFFSET = 1
HPG = NH // 2             # heads per core (2 cores per doc)
HH = HPG // 2             # heads per half (3)
RC = L // 128             # 8 chunks of 128 along c
HC = RC // 2              # ct chunks per half (4)
HA = H + 4                # hidden + ones column + pad to 772
N0 = 512                  # first PSUM bank width for the HA-dim matmuls
N1 = HA - N0              # 260
NWARM = 12                # dummy matmuls in the PE pre-warm chain
VTILE = True              # col-positioned concurrent v matmuls


def _build_nc(nsp, debug=False):
    """nsp = number of 128-row span blocks (global max over docs)."""
    import concourse.bass as bass
    import concourse.mybir as mybir
    import concourse.tile as tile
    from concourse import bacc

    f32 = mybir.dt.float32
    bf16 = mybir.dt.bfloat16
    ts, ds = bass.ts, bass.ds

    nc = bacc.Bacc("TRN2", target_bir_lowering=False, debug=debug)

    f8 = mybir.dt.float8e4
    gsp = nc.dram_tensor("gsp", [nsp * 128, HPG * L], f8, kind="ExternalInput")
    ident = nc.dram_tensor("ident", [128, 128], f8, kind="ExternalInput")
    gmt = nc.dram_tensor("gmt", [128, HPG * RC * EM], bf16, kind="ExternalInput")
    seqp = nc.dram_tensor("seqp", [128, RC * HA], bf16, kind="ExternalInput")
    wmsk = nc.dram_tensor("wmsk", [128, nsp * K + RC * K], bf16, kind="ExternalInput")
    VR = 128 if VTILE else K
    out_va = nc.dram_tensor("out_va", [VR, HA], bf16, kind="ExternalOutput")
    out_vb = nc.dram_tensor("out_vb", [VR, HA], bf16, kind="ExternalOutput")
    out_mnuma = nc.dram_tensor("out_mnuma", [EM, HA], bf16, kind="ExternalOutput")
    out_mnumb = nc.dram_tensor("out_mnumb", [EM, HA], bf16, kind="ExternalOutput")

    MHB = HPG * HC * EM       # mention cols per ct-half (3072)

    with tile.TileContext(nc) as tc:
        with (
            tc.tile_pool(name="const", bufs=1) as constp,
            tc.tile_pool(name="stream", bufs=max(3, nsp + 1)) as streamp,
            tc.tile_pool(name="tmp", bufs=2) as tmpp,
            tc.tile_pool(name="acc", bufs=1) as accp,
            tc.tile_pool(name="stage", bufs=1) as stagep,
            tc.tile_pool(name="psall", bufs=8, space="PSUM") as psall,
        ):
            # ---- scalar ring (slow): tiny consts, then the late seq half ----
            ident_s = constp.tile([128, 128], f8, name="idents")
            nc.scalar.dma_start(out=ident_s[:], in_=ident[:])
            wmsk_s = constp.tile([128, nsp * K + RC * K], bf16, name="wmsks")
            nc.scalar.dma_start(out=wmsk_s[:], in_=wmsk[:])
            seq_s = constp.tile([128, RC * HA], bf16, name="seqs")
            nc.scalar.dma_start(out=seq_s[:, HC * HA:], in_=seqp[:, HC * HA:])
            mc_of = nsp * K            # maskc block offset inside wmsk

            # ---- sync ring: early seq half, span blocks, mention ct-halves ----
            nc.sync.dma_start(out=seq_s[:, 0:HC * HA], in_=seqp[:, 0:HC * HA])
            gsp_t = []
            for sc in range(nsp):
                t = streamp.tile([128, HPG * L], f8, tag="gsp", name=f"gsp{sc}")
                nc.sync.dma_start(out=t[:], in_=gsp[ts(sc, 128), :])
                gsp_t.append(t)
            gmt_t = streamp.tile([128, HPG * RC * EM], bf16, tag="gmt", name="gmt")
            nc.sync.dma_start(out=gmt_t[:, 0:MHB], in_=gmt[:, 0:MHB])
            nc.sync.dma_start(out=gmt_t[:, MHB:], in_=gmt[:, MHB:])

            # ---- span head-sums on the PE: identity-matmul accumulation over the
            #      six heads (fp8 stream; PE idles during the stream anyway and
            #      this keeps its HAM clock warm -- replaces the dummy chain) ----
            hss = []
            for sc in range(nsp):
                h = accp.tile([128, L], bf16, tag=f"hss{sc}", name=f"hss{sc}")
                for half in range(2):
                    phs = psall.tile([128, N0], f32, tag="ps", name=f"hs{sc}{half}")
                    for hh in range(HPG):
                        nc.tensor.matmul(phs[:], ident_s[:],
                                         gsp_t[sc][:, ds(hh * L + half * N0, N0)],
                                         start=(hh == 0), stop=(hh == HPG - 1))
                    nc.vector.tensor_copy(h[:, ds(half * N0, N0)], phs[:])
                hss.append(h)

            # ---- u^T[c,k]: 16 single-shot matmuls into ONE psum bank ----
            pgs = psall.tile([128, nsp * RC * K], f32, tag="ps", name="pgs")

            for sc in range(nsp):
                for ct in range(RC):
                    nc.tensor.matmul(pgs[:, ds(sc * RC * K + ct * K, K)],
                                     hss[sc][:, ts(ct, 128)],
                                     wmsk_s[:, ds(sc * K, K)], start=True, stop=True)

            # ---- mention branch first: its inputs land before wv's, so its adds
            #      must precede the wv combine in the DVE queue and its context
            #      chains precede v in the PE queue.  Two independent ct-half
            #      chains ship on different rings as soon as each stops. ----
            mouts = (out_mnuma, out_mnumb)
            for cth in range(2):
                base = cth * MHB
                m01 = tmpp.tile([128, HC * EM], bf16, tag="ma", name="m01")
                m23 = tmpp.tile([128, HC * EM], bf16, tag="mb", name="m23")
                m45 = tmpp.tile([128, HC * EM], bf16, tag="mc", name="m45")
                hsm = accp.tile([128, HC * EM], bf16, tag=f"hsm{cth}", name=f"hsm{cth}")
                HB = HC * EM
                nc.vector.tensor_add(m01[:], gmt_t[:, ds(base + 0 * HB, HB)],
                                     gmt_t[:, ds(base + 1 * HB, HB)])
                nc.vector.tensor_add(m23[:], gmt_t[:, ds(base + 2 * HB, HB)],
                                     gmt_t[:, ds(base + 3 * HB, HB)])
                nc.vector.tensor_add(m45[:], gmt_t[:, ds(base + 4 * HB, HB)],
                                     gmt_t[:, ds(base + 5 * HB, HB)])
                nc.vector.tensor_add(m01[:], m01[:], m23[:])
                nc.vector.tensor_add(hsm[:], m01[:], m45[:])
                p0 = psall.tile([EM, N0], f32, tag="ps", name=f"pm{cth}0")
                p1 = psall.tile([EM, N1], f32, tag="ps", name=f"pm{cth}1")
                for j in range(HC):
                    ct = cth * HC + j
                    nc.tensor.matmul(p0[:], hsm[:, ts(j, EM)], seq_s[:, ds(ct * HA, N0)],
                                     start=(j == 0), stop=(j == HC - 1))
                    nc.tensor.matmul(p1[:], hsm[:, ts(j, EM)], seq_s[:, ds(ct * HA + N0, N1)],
                                     start=(j == 0), stop=(j == HC - 1))
                ms = stagep.tile([EM, HA], bf16, tag=f"mns{cth}", name=f"mns{cth}")
                if cth == 0:
                    nc.vector.tensor_copy(ms[:, 0:N0], p0[:])
                    nc.vector.tensor_copy(ms[:, N0:HA], p1[:])
                    nc.sync.dma_start(out=mouts[cth][:], in_=ms[:])
                else:
                    nc.scalar.copy(out=ms[:, 0:N0], in_=p0[:])
                    nc.scalar.copy(out=ms[:, N0:HA], in_=p1[:])
                    nc.scalar.dma_start(out=mouts[cth][:], in_=ms[:])

            # ---- mask + combine span blocks on DVE ----
            wv = accp.tile([128, RC * K], bf16, tag="wv", name="wv")
            if nsp == 1:
                nc.vector.tensor_mul(wv[:], pgs[:], wmsk_s[:, ds(mc_of, RC * K)])
            else:
                wparts = []
                for sc in range(nsp):
                    w = tmpp.tile([128, RC * K], bf16, tag="wpart", name=f"w{sc}")
                    nc.vector.tensor_mul(w[:], pgs[:, ds(sc * RC * K, RC * K)],
                                         wmsk_s[:, ds(mc_of, RC * K)])
                    wparts.append(w)
                nc.vector.tensor_add(wv[:], wparts[0][:], wparts[1][:])
                for sc in range(2, nsp):
                    nc.vector.tensor_add(wv[:], wv[:], wparts[sc][:])

            # ---- v: per-ct single-shot matmuls; 4 concurrent col-groups/bank ----
            pva0 = psall.tile([VR, N0], f32, tag="ps", name="pva0")
            pva1 = psall.tile([VR, N1], f32, tag="ps", name="pva1")
            pvb0 = psall.tile([VR, N0], f32, tag="ps", name="pvb0")
            pvb1 = psall.tile([VR, N1], f32, tag="ps", name="pvb1")
            if VTILE:
                for ct in range(RC):
                    p0, p1 = (pva0, pva1) if ct < HC else (pvb0, pvb1)
                    q = ct % HC
                    nc.tensor.matmul(p0[32 * q:32 * q + K, :], wv[:, ts(ct, K)],
                                     seq_s[:, ds(ct * HA, N0)], start=True, stop=True,
                                     tile_position=(0, 32 * q))
                    nc.tensor.matmul(p1[32 * q:32 * q + K, :], wv[:, ts(ct, K)],
                                     seq_s[:, ds(ct * HA + N0, N1)], start=True, stop=True,
                                     tile_position=(0, 32 * q))
            else:
                for ct in range(RC):
                    p0, p1 = (pva0, pva1) if ct < HC else (pvb0, pvb1)
                    j = ct % HC
                    nc.tensor.matmul(p0[:], wv[:, ts(ct, K)], seq_s[:, ds(ct * HA, N0)],
                                     start=(j == 0), stop=(j == HC - 1))
                    nc.tensor.matmul(p1[:], wv[:, ts(ct, K)], seq_s[:, ds(ct * HA + N0, N1)],
                                     start=(j == 0), stop=(j == HC - 1))
            va_s = stagep.tile([VR, HA], bf16, tag="vas", name="vas")
            vb_s = stagep.tile([VR, HA], bf16, tag="vbs", name="vbs")
            nc.scalar.copy(out=va_s[:, 0:N0], in_=pva0[:])
            nc.scalar.copy(out=va_s[:, N0:HA], in_=pva1[:])
            nc.scalar.dma_start(out=out_va[:], in_=va_s[:])
            nc.vector.tensor_copy(vb_s[:, 0:N0], pvb0[:])
            nc.vector.tensor_copy(vb_s[:, N0:HA], pvb1[:])
            nc.sync.dma_start(out=out_vb[:], in_=vb_s[:])

    nc.compile()
    return nc


_NC_CACHE = {}


def _get_nc(nsp=2):
    if nsp not in _NC_CACHE:
        _NC_CACHE[nsp] = _build_nc(nsp)
    return _NC_CACHE[nsp]


def _per_core_inputs(sequence_output, attention, mention_pos, link_start, link_len):
    """Returns (in_maps for 8 cores, per-doc span lengths, nsp)."""
    import ml_dtypes
    bf16 = ml_dtypes.bfloat16
    f8 = ml_dtypes.float8_e4m3
    identity = np.eye(128, dtype=np.float32).astype(f8)
    seq = np.asarray(sequence_output, dtype=np.float32)
    att = np.asarray(attention)
    mpos = np.asarray(mention_pos).astype(np.int64)
    lstart = np.asarray(link_start).astype(np.int64)
    llen = np.asarray(link_len).astype(np.int64)

    doc = []
    max_u = 1
    for b in range(B):
        pos = (mpos[b] + OFFSET).reshape(EM)
        s = lstart[b] + OFFSET
        e = lstart[b] + llen[b] + 1 + OFFSET
        srows = np.unique(np.concatenate([np.arange(si, ei) for si, ei in zip(s, e)]))
        max_u = max(max_u, len(srows))
        doc.append((pos, s, e, srows))
    nsp = (max_u + 127) // 128

    in_maps = []
    lengths = []
    for b in range(B):
        pos, s, e, srows = doc[b]
        nsr = len(srows)
        srows_p = np.zeros(nsp * 128, np.int64)
        srows_p[:nsr] = srows
        wspm = np.zeros((nsp * 128, K), np.float32)
        wspm[:nsr] = ((srows[:, None] >= s[None, :]) & (srows[:, None] < e[None, :]))
        wsp_p = wspm.reshape(nsp, 128, K).transpose(1, 0, 2).reshape(128, nsp * K)
        r = np.arange(L)
        maskc = ((r[:, None] >= s[None, :]) & (r[:, None] < e[None, :])).astype(np.float32)
        maskc_p = maskc.reshape(RC, 128, K).transpose(1, 0, 2).reshape(128, RC * K)
        wmsk = np.ascontiguousarray(
            np.concatenate([wsp_p, maskc_p], axis=1)).astype(bf16)
        seq_aug = np.concatenate(
            [seq[b], np.ones((L, 1), np.float32), np.zeros((L, HA - H - 1), np.float32)], axis=1)
        seqp = np.ascontiguousarray(
            seq_aug.reshape(RC, 128, HA).transpose(1, 0, 2).reshape(128, RC * HA)).astype(bf16)
        lengths.append((e - s).astype(np.float32))
        for g in range(2):
            hsl = slice(g * HPG, (g + 1) * HPG)
            # span rows, row-major: [sc*128+q, h*L+c]
            gspr = att[b, hsl][:, srows_p, :]                      # [HPG, nsp*128, L]
            gspx = np.ascontiguousarray(
                gspr.transpose(1, 0, 2).reshape(nsp * 128, HPG * L)).astype(f8)
            # mention rows, transposed, ct-half major: [p, (cth, h, ctq, m)]
            gmtr = att[b, hsl][:, pos, :]                          # [HPG, EM, L]
            gmtx = np.ascontiguousarray(
                gmtr.reshape(HPG, EM, 2, HC, 128).transpose(4, 2, 0, 3, 1)
                .reshape(128, HPG * RC * EM)).astype(bf16)
            in_maps.append({"gsp": gspx, "gmt": gmtx, "seqp": seqp, "wmsk": wmsk,
                            "ident": identity})
    return in_maps, lengths, nsp


def _combine(outs, lengths, sequence_output, type_table, mention_pos):
    seq = np.asarray(sequence_output, dtype=np.float32)
    mpos = np.asarray(mention_pos).astype(np.int64)
    ttab = np.asarray(type_table, dtype=np.float32)
    type_ids = np.concatenate(
        [np.zeros(E, np.int64), np.ones(EM, np.int64), np.full(K, 2, np.int64)])
    nodes_type = ttab[type_ids]  # [E+EM+K, TYPE_DIM]

    def vsum(o):
        va = o["out_va"].astype(np.float32)
        vb = o["out_vb"].astype(np.float32)
        if va.shape[0] == K:
            return va + vb
        groups = [va[32 * q:32 * q + K] for q in range(HC)] + \
                 [vb[32 * q:32 * q + K] for q in range(HC)]
        return np.sum(groups, axis=0)

    out = np.zeros((B, E + EM + K + E + EM, H + TYPE_DIM), np.float32)
    for b in range(B):
        o0, o1 = outs[2 * b], outs[2 * b + 1]
        v = vsum(o0) + vsum(o1)
        mnum = (o0["out_mnuma"].astype(np.float32) + o0["out_mnumb"].astype(np.float32)
                + o1["out_mnuma"].astype(np.float32) + o1["out_mnumb"].astype(np.float32))
        length = lengths[b]

        link_rep = v[:, :H] / (NH * length[:, None])
        m_ctx = mnum[:, :H] / (mnum[:, H:H + 1] + NH * 1e-5)
        enum = mnum.reshape(E, MPE, HA).sum(axis=1)
        e_ctx = enum[:, :H] / (enum[:, H:H + 1] + NH * MPE * 1e-5)

        pos = (mpos[b] + OFFSET).reshape(EM)
        memb = seq[b, pos]                                          # exact gather
        mg = memb.reshape(E, MPE, H)
        mmax = mg.max(axis=1)
        eemb = np.log(np.exp(mg - mmax[:, None, :]).sum(axis=1)) + mmax

        nodes_raw = np.concatenate([eemb, memb, link_rep], axis=0)  # [176,H]
        nodes = np.concatenate([nodes_raw, nodes_type], axis=1)     # [176,H+20]
        ctx = np.concatenate([e_ctx, m_ctx], axis=0)                # [160,H]
        ctx = np.concatenate([ctx, np.zeros((E + EM, TYPE_DIM), np.float32)], axis=1)
        out[b] = np.concatenate([nodes, ctx], axis=0)
    return out


def kernel(**inputs):
    from concourse.bass_utils import run_bass_kernel_spmd

    in_maps, lengths, nsp = _per_core_inputs(
        inputs["sequence_output"], inputs["attention"],
        inputs["mention_pos"], inputs["link_start"], inputs["link_len"])
    nc = _get_nc(nsp)
    res = run_bass_kernel_spmd(nc, in_maps, core_ids=list(range(8)))
    return _combine(res.results, lengths, inputs["sequence_output"],
                    inputs["type_table"], inputs["mention_pos"])
